# revision 4
# baseline (speedup 1.0000x reference)
"""Trainium2 Bass kernel for the BiDAF-style AttentionFlow layer.

Reference math (per example b):
    S[i,j]  = c_i.w_c + q_j.w_q + (c_i*w_cq).q_j          (400, 50)
    S1      = softmax_i(S); S2 = softmax_j(S)
    A       = S1 @ q                                       (400, 800)
    Bv      = (S1 @ S2^T) @ c  ==  S1 @ (S2^T @ c)         (400, 800)
    out     = [c, A, c*A, c*Bv]                            (400, 3200)

Key restructurings:
  * Bv is reassociated to S1 @ (S2^T @ c): the (400,400) M matrix is never
    materialized and the 400x400x800 matmul collapses to 50x400x800.
  * S is computed transposed (S^T: 50 part x 400 free) in one PSUM pass with
    the sc[i] / sq[j] rank-1 bias terms added via two K=1 augmentation
    matmuls, so softmax over i is a free-dim softmax; softmax over j is done
    after four 50x100 PE transposes.

Data-parallel over batch: 32 examples -> 8 NeuronCores x 4 examples.
"""

import numpy as np

import concourse.bass as bass
import concourse.tile as tile
from concourse import bacc, mybir
from concourse.bass_utils import run_bass_kernel_spmd
from concourse.masks import make_identity

F32 = mybir.dt.float32

B, CLEN, QLEN, D = 32, 400, 50, 800
NCORES = 8
BSH = B // NCORES          # 4 examples per core
IC = 4                     # i-chunks over CLEN
IP = CLEN // IC            # 100 rows per i-chunk
DC = 7                     # d-chunks over D (6*128 + 32)
DSZ = [128] * 6 + [32]
NH = 2                     # n-halves over D for PSUM-bank-sized matmuls
HALF = D // NH             # 400


def build_nc():
    nc = bacc.Bacc("TRN2", target_bir_lowering=False, debug=False, num_devices=NCORES)
    c = nc.dram_tensor("c", [BSH, CLEN, D], F32, kind="ExternalInput").ap()
    q = nc.dram_tensor("q", [BSH, QLEN, D], F32, kind="ExternalInput").ap()
    w0 = nc.dram_tensor("W0", [3 * D], F32, kind="ExternalInput").ap()
    out = nc.dram_tensor("out", [BSH, CLEN, 4 * D], F32, kind="ExternalOutput").ap()

    with tile.TileContext(nc) as tc:
        with (
            tc.tile_pool(name="singles", bufs=1) as singles,
            tc.tile_pool(name="ex", bufs=2) as ex,
            tc.tile_pool(name="osb", bufs=3) as osb_pool,
            tc.tile_pool(name="ps_small", bufs=2, space="PSUM") as ps_small,
            tc.tile_pool(name="ps_big", bufs=6, space="PSUM") as ps_big,
        ):
            # ---------------- one-time setup ----------------
            ident = singles.tile([128, 128], F32)
            make_identity(nc, ident)

            w0row = singles.tile([1, 3 * D], F32)
            nc.sync.dma_start(out=w0row, in_=w0.rearrange("(p f) -> p f", p=1))

            ones50 = singles.tile([1, QLEN], F32)
            nc.vector.memset(ones50, 1.0)
            ones_c = singles.tile([1, CLEN], F32)
            nc.vector.memset(ones_c, 1.0)

            # w_c / w_q / w_cq as per-partition columns: wcol[:, t*7+k]
            # holds W0[t*800 + k*128 : ...] transposed onto partitions.
            wps = ps_small.tile([128, 21], F32, tag="ps_small")
            for t in range(3):
                for k in range(DC):
                    sz = DSZ[k]
                    nc.tensor.transpose(
                        out=wps[0:sz, t * DC + k : t * DC + k + 1],
                        in_=w0row[0:1, t * D + k * 128 : t * D + k * 128 + sz],
                        identity=ident[0:1, 0:1],
                    )
            wcol = singles.tile([128, 21], F32)
            for t in range(3):
                nc.scalar.copy(
                    out=wcol[:, t * 7 : t * 7 + 6], in_=wps[:, t * 7 : t * 7 + 6]
                )
                nc.scalar.copy(
                    out=wcol[0:32, t * 7 + 6 : t * 7 + 7],
                    in_=wps[0:32, t * 7 + 6 : t * 7 + 7],
                )

            # ---------------- per-example pipeline ----------------
            for b in range(BSH):
                c_nat = ex.tile([IP, IC, D], F32)   # c[b] natural, i-chunked
                q_nat = ex.tile([QLEN, D], F32)
                nc.sync.dma_start(
                    out=c_nat, in_=c[b].rearrange("(ch p) d -> p ch d", p=IP)
                )
                nc.sync.dma_start(out=q_nat, in_=q[b])

                # ---- q^T (d-major) via PE transposes ----
                qt_ps = ps_small.tile([128, DC, QLEN], F32, tag="ps_small")
                for k in range(DC):
                    sz = DSZ[k]
                    nc.tensor.transpose(
                        out=qt_ps[0:sz, k, :],
                        in_=q_nat[:, k * 128 : k * 128 + sz],
                        identity=ident[0:QLEN, 0:QLEN],
                    )
                qT = ex.tile([128, DC, QLEN], F32)
                nc.scalar.copy(out=qT[:, 0:6, :], in_=qt_ps[:, 0:6, :])
                nc.scalar.copy(out=qT[0:32, 6, :], in_=qt_ps[0:32, 6, :])

                # qw^T = q^T * w_cq (per-partition scalars)
                qwT = ex.tile([128, DC, QLEN], F32)
                for k in range(DC):
                    sz = DSZ[k]
                    nc.vector.tensor_scalar_mul(
                        qwT[0:sz, k, :], qT[0:sz, k, :], wcol[0:sz, 14 + k : 15 + k]
                    )

                # ---- c^T (d-major) via PE transposes ----
                cT = ex.tile([128, DC, CLEN], F32)
                for k in range(DC):
                    sz = DSZ[k]
                    ct_ps = ps_small.tile([128, CLEN], F32, tag="ps_small")
                    for ch in range(IC):
                        nc.tensor.transpose(
                            out=ct_ps[0:sz, ch * IP : (ch + 1) * IP],
                            in_=c_nat[:, ch, k * 128 : k * 128 + sz],
                            identity=ident[0:IP, 0:IP],
                        )
                    nc.scalar.copy(out=cT[0:sz, k, :], in_=ct_ps[0:sz, :])

                # ---- sc[i] = c_i.w_c  (1,400);  sq[j] = q_j.w_q  (1,50) ----
                sc_ps = ps_small.tile([1, CLEN], F32, tag="ps_small")
                for k in range(DC):
                    sz = DSZ[k]
                    nc.tensor.matmul(
                        out=sc_ps,
                        lhsT=wcol[0:sz, k : k + 1],
                        rhs=cT[0:sz, k, :],
                        start=(k == 0),
                        stop=(k == DC - 1),
                    )
                sc_row = ex.tile([1, CLEN], F32)
                nc.scalar.copy(out=sc_row, in_=sc_ps)

                sq_ps = ps_small.tile([1, QLEN], F32, tag="ps_small")
                for k in range(DC):
                    sz = DSZ[k]
                    nc.tensor.matmul(
                        out=sq_ps,
                        lhsT=wcol[0:sz, 7 + k : 8 + k],
                        rhs=qT[0:sz, k, :],
                        start=(k == 0),
                        stop=(k == DC - 1),
                    )
                sq_row = ex.tile([1, QLEN], F32)
                nc.scalar.copy(out=sq_row, in_=sq_ps)

                # ---- S^T = qw^T.T @ c^T + ones.sc + sq.ones  (50, 400) ----
                st_ps = ps_small.tile([QLEN, CLEN], F32, tag="ps_small")
                for k in range(DC):
                    sz = DSZ[k]
                    nc.tensor.matmul(
                        out=st_ps,
                        lhsT=qwT[0:sz, k, :],
                        rhs=cT[0:sz, k, :],
                        start=(k == 0),
                        stop=False,
                    )
                nc.tensor.matmul(
                    out=st_ps, lhsT=ones50, rhs=sc_row, start=False, stop=False
                )
                nc.tensor.matmul(
                    out=st_ps, lhsT=sq_row, rhs=ones_c, start=False, stop=True
                )
                St = ex.tile([QLEN, CLEN], F32)
                nc.scalar.copy(out=St, in_=st_ps)

                # ---- S1^T = softmax over free dim of S^T ----
                m1 = ex.tile([QLEN, 1], F32)
                nc.vector.tensor_reduce(
                    out=m1, in_=St, axis=mybir.AxisListType.X,
                    op=mybir.AluOpType.max, negate=True,
                )
                e1 = ex.tile([QLEN, CLEN], F32)
                s1sum = ex.tile([QLEN, 1], F32)
                nc.scalar.activation(
                    out=e1, in_=St, func=mybir.ActivationFunctionType.Exp,
                    bias=m1, scale=1.0, accum_out=s1sum,
                )
                r1 = ex.tile([QLEN, 1], F32)
                nc.vector.reciprocal(out=r1, in_=s1sum)
                nc.vector.tensor_scalar_mul(e1, e1, r1)  # e1 := S1^T

                # ---- S natural (PE transpose) + softmax over j -> S2 ----
                sn_ps = ps_small.tile([IP, IC, QLEN], F32, tag="ps_small")
                for ch in range(IC):
                    nc.tensor.transpose(
                        out=sn_ps[:, ch, :],
                        in_=St[:, ch * IP : (ch + 1) * IP],
                        identity=ident[0:QLEN, 0:QLEN],
                    )
                s2 = ex.tile([IP, IC, QLEN], F32)
                m2 = ex.tile([IP, IC], F32)
                s2sum = ex.tile([IP, IC], F32)
                r2 = ex.tile([IP, IC], F32)
                for ch in range(IC):
                    nc.vector.tensor_reduce(
                        out=m2[:, ch : ch + 1], in_=sn_ps[:, ch, :],
                        axis=mybir.AxisListType.X, op=mybir.AluOpType.max,
                        negate=True,
                    )
                    nc.scalar.activation(
                        out=s2[:, ch, :], in_=sn_ps[:, ch, :],
                        func=mybir.ActivationFunctionType.Exp,
                        bias=m2[:, ch : ch + 1], scale=1.0,
                        accum_out=s2sum[:, ch : ch + 1],
                    )
                    nc.vector.reciprocal(
                        out=r2[:, ch : ch + 1], in_=s2sum[:, ch : ch + 1]
                    )
                    nc.vector.tensor_scalar_mul(
                        s2[:, ch, :], s2[:, ch, :], r2[:, ch : ch + 1]
                    )

                # ---- T = S2^T @ c  (50, 800) ----
                T_sb = ex.tile([QLEN, D], F32)
                for h in range(NH):
                    t_ps = ps_big.tile([IP, HALF], F32, tag="ps_big")
                    for ch in range(IC):
                        nc.tensor.matmul(
                            out=t_ps[0:QLEN, :],
                            lhsT=s2[:, ch, :],
                            rhs=c_nat[:, ch, h * HALF : (h + 1) * HALF],
                            start=(ch == 0),
                            stop=(ch == IC - 1),
                        )
                    nc.scalar.copy(
                        out=T_sb[:, h * HALF : (h + 1) * HALF], in_=t_ps[0:QLEN, :]
                    )

                # ---- per i-chunk: A, Bv, elementwise, store ----
                for ch in range(IC):
                    o_sb = osb_pool.tile([IP, 3 * D], F32, tag="osb")
                    a_ps = []
                    bv_ps = []
                    for h in range(NH):
                        ap = ps_big.tile([IP, HALF], F32, tag="ps_big")
                        nc.tensor.matmul(
                            out=ap,
                            lhsT=e1[:, ch * IP : (ch + 1) * IP],
                            rhs=q_nat[:, h * HALF : (h + 1) * HALF],
                            start=True, stop=True,
                        )
                        a_ps.append(ap)
                    for h in range(NH):
                        bp = ps_big.tile([IP, HALF], F32, tag="ps_big")
                        nc.tensor.matmul(
                            out=bp,
                            lhsT=e1[:, ch * IP : (ch + 1) * IP],
                            rhs=T_sb[:, h * HALF : (h + 1) * HALF],
                            start=True, stop=True,
                        )
                        bv_ps.append(bp)
                    for h in range(NH):
                        sl = slice(h * HALF, (h + 1) * HALF)
                        nc.scalar.copy(out=o_sb[:, h * HALF : (h + 1) * HALF], in_=a_ps[h])
                        nc.vector.tensor_mul(
                            o_sb[:, D + h * HALF : D + (h + 1) * HALF],
                            c_nat[:, ch, sl], a_ps[h],
                        )
                        nc.vector.tensor_mul(
                            o_sb[:, 2 * D + h * HALF : 2 * D + (h + 1) * HALF],
                            c_nat[:, ch, sl], bv_ps[h],
                        )
                    nc.sync.dma_start(
                        out=out[b, ch * IP : (ch + 1) * IP, 0:D], in_=c_nat[:, ch, :]
                    )
                    nc.sync.dma_start(
                        out=out[b, ch * IP : (ch + 1) * IP, D : 4 * D], in_=o_sb
                    )

    nc.compile()
    return nc


_NC = None


def _get_nc():
    global _NC
    if _NC is None:
        _NC = build_nc()
    return _NC


def run(inputs: dict, trace: bool = False, **kw):
    nc = _get_nc()
    c, q, w0 = inputs["c"], inputs["q"], inputs["W0"]
    in_maps = [
        {
            "c": np.ascontiguousarray(c[i * BSH : (i + 1) * BSH]),
            "q": np.ascontiguousarray(q[i * BSH : (i + 1) * BSH]),
            "W0": np.ascontiguousarray(w0),
        }
        for i in range(NCORES)
    ]
    res = run_bass_kernel_spmd(
        nc, in_maps, core_ids=list(range(NCORES)), trace=trace, **kw
    )
    outs = np.concatenate([r["out"] for r in res.results], axis=0)
    return outs, res


def kernel(**inputs) -> np.ndarray:
    out, _ = run(inputs, trace=False)
    return out


# revision 17
# speedup vs baseline: 1.0720x; 1.0720x over previous
"""Trainium2 Bass kernel for the BiDAF-style AttentionFlow layer.

Reference math (per example b):
    S[i,j]  = c_i.w_c + q_j.w_q + (c_i*w_cq).q_j          (400, 50)
    S1      = softmax_i(S); S2 = softmax_j(S)
    A       = S1 @ q                                       (400, 800)
    Bv      = (S1 @ S2^T) @ c  ==  S1 @ (S2^T @ c)         (400, 800)
    out     = [c, A, c*A, c*Bv]                            (400, 3200)

Key restructurings:
  * Bv is reassociated to S1 @ (S2^T @ c): the (400,400) M matrix is never
    materialized and the 400x400x800 matmul collapses to 50x400x800.
  * S is computed transposed (S^T: 51 part x 400 free) in one PSUM pass;
    the stationary is [qw | w_c] so PSUM row 50 is sc = c.w_c for free, and
    the sc[i] / sq[j] rank-1 bias terms are added with two K=1 matmuls.
    Softmax over i is then a free-dim softmax; softmax over j happens after
    four 50x100 PE transposes.
  * All big matmuls run in float32r (single-pass PE streaming, 4x faster
    than fp32); operands live in f32r-typed tiles so producers round.

Data-parallel over batch: 32 examples -> 8 NeuronCores x 4 examples.
"""

import numpy as np

import concourse.bass as bass
import concourse.tile as tile
from concourse import bacc, mybir
from concourse.bass_utils import run_bass_kernel_spmd
from concourse.masks import make_identity

F32 = mybir.dt.float32
R32 = mybir.dt.float32r  # fp32 storage, TF32-like single-pass PE matmul

B, CLEN, QLEN, D = 32, 400, 50, 800
NCORES = 8
BSH = B // NCORES          # 4 examples per core
IC = 4                     # i-chunks over CLEN
IP = CLEN // IC            # 100 rows per i-chunk
DC = 7                     # d-chunks over D (6*128 + 32)
DSZ = [128] * 6 + [32]
NH = 2                     # n-halves over D for PSUM-bank-sized matmuls
HALF = D // NH             # 400


def build_nc():
    nc = bacc.Bacc("TRN2", target_bir_lowering=False, debug=False, num_devices=NCORES)
    c = nc.dram_tensor("c", [BSH, CLEN, D], F32, kind="ExternalInput").ap()
    q = nc.dram_tensor("q", [BSH, QLEN, D], F32, kind="ExternalInput").ap()
    w0 = nc.dram_tensor("W0", [3 * D], F32, kind="ExternalInput").ap()
    out = nc.dram_tensor("out", [BSH, CLEN, 4 * D], F32, kind="ExternalOutput").ap()

    with tile.TileContext(nc) as tc:
        with (
            tc.tile_pool(name="singles", bufs=1) as singles,
            tc.tile_pool(name="ex", bufs=2) as ex,
            tc.tile_pool(name="osb", bufs=3) as osb_pool,
            tc.tile_pool(name="ps_small", bufs=2, space="PSUM") as ps_small,
            tc.tile_pool(name="ps_big", bufs=6, space="PSUM") as ps_big,
        ):
            # ---------------- one-time setup ----------------
            ident = singles.tile([128, 128], F32)
            make_identity(nc, ident)

            w0row = singles.tile([1, 3 * D], F32)
            nc.sync.dma_start(out=w0row, in_=w0.rearrange("(p f) -> p f", p=1))

            ones_f = singles.tile([1, CLEN], F32)
            nc.vector.memset(ones_f, 1.0)
            ones50 = singles.tile([1, QLEN], R32)
            nc.scalar.copy(out=ones50, in_=ones_f[:, 0:QLEN])
            ones_c = singles.tile([1, CLEN], R32)
            nc.scalar.copy(out=ones_c, in_=ones_f)
            zeros_f = singles.tile([128, 14], F32)
            nc.vector.memset(zeros_f, 0.0)

            # w_c / w_q / w_cq as per-partition columns: wcol[:, t*7+k]
            # holds W0[t*800 + k*128 : ...] transposed onto partitions.
            wps = ps_small.tile([128, 21], F32, tag="ps_small")
            for t in range(3):
                for k in range(DC):
                    sz = DSZ[k]
                    nc.tensor.transpose(
                        out=wps[0:sz, t * DC + k : t * DC + k + 1],
                        in_=w0row[0:1, t * D + k * 128 : t * D + k * 128 + sz],
                        identity=ident[0:1, 0:1],
                    )
            wcol = singles.tile([128, 21], F32)
            wcol_r = singles.tile([128, 21], R32)
            for w_t in (wcol, wcol_r):
                for t in range(3):
                    nc.scalar.copy(
                        out=w_t[:, t * 7 : t * 7 + 6], in_=wps[:, t * 7 : t * 7 + 6]
                    )
                    nc.scalar.copy(
                        out=w_t[0:32, t * 7 + 6 : t * 7 + 7],
                        in_=wps[0:32, t * 7 + 6 : t * 7 + 7],
                    )

            # ---------------- per-example pipeline ----------------
            for b in range(BSH):
                c_nat = ex.tile([IP, IC, D], F32)   # c[b] natural, i-chunked
                q_nat = ex.tile([QLEN, D], F32)
                nc.sync.dma_start(
                    out=c_nat, in_=c[b].rearrange("(ch p) d -> p ch d", p=IP)
                )
                nc.sync.dma_start(out=q_nat, in_=q[b])

                # rounded copies for f32r matmul rhs use
                c_r = ex.tile([IP, IC, D], R32)
                nc.gpsimd.tensor_copy(out=c_r, in_=c_nat)
                q_r = ex.tile([QLEN, D], R32)
                nc.gpsimd.tensor_copy(out=q_r, in_=q_nat)

                # ---- q^T (d-major) via PE transposes ----
                qt_ps = ps_small.tile([128, DC, QLEN], F32, tag="ps_small")
                for k in range(DC):
                    sz = DSZ[k]
                    nc.tensor.transpose(
                        out=qt_ps[0:sz, k, :],
                        in_=q_nat[:, k * 128 : k * 128 + sz],
                        identity=ident[0:QLEN, 0:QLEN],
                    )
                qT = ex.tile([128, DC, QLEN], R32)
                nc.scalar.copy(out=qT[:, 0:6, :], in_=qt_ps[:, 0:6, :])
                nc.scalar.copy(out=qT[0:32, 6, :], in_=qt_ps[0:32, 6, :])

                # qw^T = q^T * w_cq (per-partition scalars); column 64 holds
                # w_c so the S~ matmul also emits sc = c.w_c as PSUM row 64
                # (engine PSUM reads must start at a multiple-of-32 partition).
                qwT = ex.tile([128, DC, 65], R32)
                for k in range(DC):
                    sz = DSZ[k]
                    nc.vector.tensor_scalar_mul(
                        qwT[0:sz, k, 0:QLEN], qT[0:sz, k, :], wcol[0:sz, 14 + k : 15 + k]
                    )
                    nc.gpsimd.tensor_copy(
                        out=qwT[0:sz, k, QLEN:64], in_=zeros_f[0:sz, :]
                    )
                    nc.gpsimd.tensor_copy(
                        out=qwT[0:sz, k, 64:65], in_=wcol[0:sz, k : k + 1]
                    )

                # ---- c^T (d-major) via PE transposes ----
                cT = ex.tile([128, DC, CLEN], R32)
                for k in range(DC):
                    sz = DSZ[k]
                    ct_ps = ps_small.tile([128, CLEN], F32, tag="ps_small")
                    for ch in range(IC):
                        nc.tensor.transpose(
                            out=ct_ps[0:sz, ch * IP : (ch + 1) * IP],
                            in_=c_nat[:, ch, k * 128 : k * 128 + sz],
                            identity=ident[0:IP, 0:IP],
                        )
                    nc.scalar.copy(out=cT[0:sz, k, :], in_=ct_ps[0:sz, :])

                # ---- sq[j] = q_j.w_q  (1,50) ----
                sq_ps = ps_small.tile([1, QLEN], F32, tag="ps_small")
                for k in range(DC):
                    sz = DSZ[k]
                    nc.tensor.matmul(
                        out=sq_ps,
                        lhsT=wcol_r[0:sz, 7 + k : 8 + k],
                        rhs=qT[0:sz, k, :],
                        start=(k == 0),
                        stop=(k == DC - 1),
                    )
                sq_row = ex.tile([1, QLEN], R32)
                nc.scalar.copy(out=sq_row, in_=sq_ps)

                # ---- S^T (+ sc row 64) = [qw|0|w_c]^T.T @ c^T  (65, 400) ----
                st_ps = ps_small.tile([65, CLEN], F32, tag="ps_small")
                for k in range(DC):
                    sz = DSZ[k]
                    nc.tensor.matmul(
                        out=st_ps,
                        lhsT=qwT[0:sz, k, :],
                        rhs=cT[0:sz, k, :],
                        start=(k == 0),
                        stop=(k == DC - 1),
                    )
                sc_row = ex.tile([1, CLEN], R32)
                nc.scalar.copy(out=sc_row, in_=st_ps[64:65, :])
                # rank-1 bias terms accumulated onto the closed S~ group
                nc.tensor.matmul(
                    out=st_ps[0:QLEN, :], lhsT=ones50, rhs=sc_row,
                    start=False, stop=False, skip_group_check=True,
                )
                nc.tensor.matmul(
                    out=st_ps[0:QLEN, :], lhsT=sq_row, rhs=ones_c,
                    start=False, stop=True, skip_group_check=True,
                )
                St = ex.tile([QLEN, CLEN], F32)
                nc.scalar.copy(out=St, in_=st_ps[0:QLEN, :])

                # ---- S1^T = softmax over free dim of S^T ----
                m1 = ex.tile([QLEN, 1], F32)
                nc.vector.tensor_reduce(
                    out=m1, in_=St, axis=mybir.AxisListType.X,
                    op=mybir.AluOpType.max, negate=True,
                )
                e1 = ex.tile([QLEN, CLEN], R32)
                s1sum = ex.tile([QLEN, 1], F32)
                nc.scalar.activation(
                    out=e1, in_=St, func=mybir.ActivationFunctionType.Exp,
                    bias=m1, scale=1.0, accum_out=s1sum,
                )
                r1 = ex.tile([QLEN, 1], F32)
                nc.vector.reciprocal(out=r1, in_=s1sum)
                nc.vector.tensor_scalar_mul(e1, e1, r1)  # e1 := S1^T

                # ---- S natural (PE transpose) + softmax over j -> S2 ----
                sn_ps = ps_small.tile([IP, IC, QLEN], F32, tag="ps_small")
                for ch in range(IC):
                    nc.tensor.transpose(
                        out=sn_ps[:, ch, :],
                        in_=St[:, ch * IP : (ch + 1) * IP],
                        identity=ident[0:QLEN, 0:QLEN],
                    )
                s2 = ex.tile([IP, IC, QLEN], R32)
                m2 = ex.tile([IP, IC], F32)
                s2sum = ex.tile([IP, IC], F32)
                r2 = ex.tile([IP, IC], F32)
                for ch in range(IC):
                    nc.vector.tensor_reduce(
                        out=m2[:, ch : ch + 1], in_=sn_ps[:, ch, :],
                        axis=mybir.AxisListType.X, op=mybir.AluOpType.max,
                        negate=True,
                    )
                    nc.scalar.activation(
                        out=s2[:, ch, :], in_=sn_ps[:, ch, :],
                        func=mybir.ActivationFunctionType.Exp,
                        bias=m2[:, ch : ch + 1], scale=1.0,
                        accum_out=s2sum[:, ch : ch + 1],
                    )
                    nc.vector.reciprocal(
                        out=r2[:, ch : ch + 1], in_=s2sum[:, ch : ch + 1]
                    )
                    nc.vector.tensor_scalar_mul(
                        s2[:, ch, :], s2[:, ch, :], r2[:, ch : ch + 1]
                    )

                # ---- T = S2^T @ c  (50, 800) ----
                T_sb = ex.tile([QLEN, D], R32)
                for h in range(NH):
                    t_ps = ps_big.tile([IP, HALF], F32, tag="ps_big")
                    for ch in range(IC):
                        nc.tensor.matmul(
                            out=t_ps[0:QLEN, :],
                            lhsT=s2[:, ch, :],
                            rhs=c_r[:, ch, h * HALF : (h + 1) * HALF],
                            start=(ch == 0),
                            stop=(ch == IC - 1),
                        )
                    nc.scalar.copy(
                        out=T_sb[:, h * HALF : (h + 1) * HALF], in_=t_ps[0:QLEN, :]
                    )

                # ---- per i-chunk: A, Bv, elementwise, store ----
                for ch in range(IC):
                    o_sb = osb_pool.tile([IP, 3 * D], F32, tag="osb")
                    a_ps = []
                    bv_ps = []
                    for h in range(NH):
                        ap = ps_big.tile([IP, HALF], F32, tag="ps_big")
                        nc.tensor.matmul(
                            out=ap,
                            lhsT=e1[:, ch * IP : (ch + 1) * IP],
                            rhs=q_r[:, h * HALF : (h + 1) * HALF],
                            start=True, stop=True,
                        )
                        a_ps.append(ap)
                    for h in range(NH):
                        bp = ps_big.tile([IP, HALF], F32, tag="ps_big")
                        nc.tensor.matmul(
                            out=bp,
                            lhsT=e1[:, ch * IP : (ch + 1) * IP],
                            rhs=T_sb[:, h * HALF : (h + 1) * HALF],
                            start=True, stop=True,
                        )
                        bv_ps.append(bp)
                    for h in range(NH):
                        sl = slice(h * HALF, (h + 1) * HALF)
                        nc.scalar.copy(out=o_sb[:, h * HALF : (h + 1) * HALF], in_=a_ps[h])
                        nc.vector.tensor_mul(
                            o_sb[:, D + h * HALF : D + (h + 1) * HALF],
                            c_nat[:, ch, sl], a_ps[h],
                        )
                        nc.vector.tensor_mul(
                            o_sb[:, 2 * D + h * HALF : 2 * D + (h + 1) * HALF],
                            c_nat[:, ch, sl], bv_ps[h],
                        )
                    nc.gpsimd.dma_start(
                        out=out[b, ch * IP : (ch + 1) * IP, 0:D], in_=c_nat[:, ch, :]
                    )
                    nc.gpsimd.dma_start(
                        out=out[b, ch * IP : (ch + 1) * IP, D : 4 * D], in_=o_sb
                    )

    nc.compile()
    return nc


_NC = None


def _get_nc():
    global _NC
    if _NC is None:
        _NC = build_nc()
    return _NC


def run(inputs: dict, trace: bool = False, **kw):
    nc = _get_nc()
    c, q, w0 = inputs["c"], inputs["q"], inputs["W0"]
    in_maps = [
        {
            "c": np.ascontiguousarray(c[i * BSH : (i + 1) * BSH]),
            "q": np.ascontiguousarray(q[i * BSH : (i + 1) * BSH]),
            "W0": np.ascontiguousarray(w0),
        }
        for i in range(NCORES)
    ]
    res = run_bass_kernel_spmd(
        nc, in_maps, core_ids=list(range(NCORES)), trace=trace, **kw
    )
    outs = np.concatenate([r["out"] for r in res.results], axis=0)
    return outs, res


def kernel(**inputs) -> np.ndarray:
    out, _ = run(inputs, trace=False)
    return out


# revision 20
# speedup vs baseline: 1.2923x; 1.2055x over previous
"""Trainium2 Bass kernel for the BiDAF-style AttentionFlow layer.

Reference math (per example b):
    S[i,j]  = c_i.w_c + q_j.w_q + (c_i*w_cq).q_j          (400, 50)
    S1      = softmax_i(S); S2 = softmax_j(S)
    A       = S1 @ q                                       (400, 800)
    Bv      = (S1 @ S2^T) @ c  ==  S1 @ (S2^T @ c)         (400, 800)
    out     = [c, A, c*A, c*Bv]                            (400, 3200)

Key restructurings:
  * Bv is reassociated to S1 @ (S2^T @ c): the (400,400) M matrix is never
    materialized and the 400x400x800 matmul collapses to 50x400x800.
  * S is computed transposed (S^T: 51 part x 400 free) in one PSUM pass;
    the stationary is [qw | w_c] so PSUM row 50 is sc = c.w_c for free, and
    the sc[i] / sq[j] rank-1 bias terms are added with two K=1 matmuls.
    Softmax over i is then a free-dim softmax; softmax over j happens after
    four 50x100 PE transposes.
  * All big matmuls run in float32r (single-pass PE streaming, 4x faster
    than fp32); operands live in f32r-typed tiles so producers round.

Data-parallel over batch: 32 examples -> 8 NeuronCores x 4 examples.
"""

import numpy as np

import concourse.bass as bass
import concourse.tile as tile
from concourse import bacc, mybir
from concourse.bass_utils import run_bass_kernel_spmd
from concourse.masks import make_identity

F32 = mybir.dt.float32
R32 = mybir.dt.float32r  # fp32 storage, TF32-like single-pass PE matmul

B, CLEN, QLEN, D = 32, 400, 50, 800
NCORES = 8
BSH = B // NCORES          # 4 examples per core
IC = 4                     # i-chunks over CLEN
IP = CLEN // IC            # 100 rows per i-chunk
DC = 7                     # d-chunks over D (6*128 + 32)
DSZ = [128] * 6 + [32]
NH = 2                     # n-halves over D for PSUM-bank-sized matmuls
HALF = D // NH             # 400


def build_nc():
    nc = bacc.Bacc("TRN2", target_bir_lowering=False, debug=False, num_devices=NCORES)
    c = nc.dram_tensor("c", [BSH, CLEN, D], F32, kind="ExternalInput").ap()
    q = nc.dram_tensor("q", [BSH, QLEN, D], F32, kind="ExternalInput").ap()
    w0 = nc.dram_tensor("W0", [3 * D], F32, kind="ExternalInput").ap()
    out = nc.dram_tensor("out", [BSH, CLEN, 4 * D], F32, kind="ExternalOutput").ap()

    with tile.TileContext(nc) as tc:
        with (
            tc.tile_pool(name="singles", bufs=1) as singles,
            tc.tile_pool(name="ex", bufs=2) as ex,
            tc.tile_pool(name="osb", bufs=3) as osb_pool,
            tc.tile_pool(name="ps_small", bufs=2, space="PSUM") as ps_small,
            tc.tile_pool(name="ps_big", bufs=6, space="PSUM") as ps_big,
        ):
            # ---------------- one-time setup ----------------
            ident = singles.tile([128, 128], F32)
            make_identity(nc, ident)

            w0row = singles.tile([1, 3 * D], F32)
            nc.sync.dma_start(out=w0row, in_=w0.rearrange("(p f) -> p f", p=1))

            ones_f = singles.tile([1, CLEN], F32)
            nc.vector.memset(ones_f, 1.0)
            ones50 = singles.tile([1, QLEN], R32)
            nc.scalar.copy(out=ones50, in_=ones_f[:, 0:QLEN])
            ones_c = singles.tile([1, CLEN], R32)
            nc.scalar.copy(out=ones_c, in_=ones_f)
            zeros_f = singles.tile([128, 14], F32)
            nc.vector.memset(zeros_f, 0.0)

            # w_c / w_q / w_cq as per-partition columns: wcol[:, t*7+k]
            # holds W0[t*800 + k*128 : ...] transposed onto partitions.
            wps = ps_small.tile([128, 21], F32, tag="ps_small")
            for t in range(3):
                for k in range(DC):
                    sz = DSZ[k]
                    nc.tensor.transpose(
                        out=wps[0:sz, t * DC + k : t * DC + k + 1],
                        in_=w0row[0:1, t * D + k * 128 : t * D + k * 128 + sz],
                        identity=ident[0:1, 0:1],
                    )
            wcol = singles.tile([128, 21], F32)
            wcol_r = singles.tile([128, 21], R32)
            for w_t in (wcol, wcol_r):
                for t in range(3):
                    nc.scalar.copy(
                        out=w_t[:, t * 7 : t * 7 + 6], in_=wps[:, t * 7 : t * 7 + 6]
                    )
                    nc.scalar.copy(
                        out=w_t[0:32, t * 7 + 6 : t * 7 + 7],
                        in_=wps[0:32, t * 7 + 6 : t * 7 + 7],
                    )

            # ---------------- per-example pipeline ----------------
            for b in range(BSH):
                c_nat = ex.tile([IP, IC, D], F32)   # c[b] natural, i-chunked
                q_nat = ex.tile([QLEN, D], F32)
                nc.sync.dma_start(
                    out=c_nat, in_=c[b].rearrange("(ch p) d -> p ch d", p=IP)
                )
                nc.sync.dma_start(out=q_nat, in_=q[b])

                # ---- q^T (d-major) via PE transposes ----
                qt_ps = ps_small.tile([128, DC, QLEN], F32, tag="ps_small")
                for k in range(DC):
                    sz = DSZ[k]
                    nc.tensor.transpose(
                        out=qt_ps[0:sz, k, :],
                        in_=q_nat[:, k * 128 : k * 128 + sz],
                        identity=ident[0:QLEN, 0:QLEN],
                    )
                qT = ex.tile([128, DC, QLEN], R32)
                nc.scalar.copy(out=qT[:, 0:6, :], in_=qt_ps[:, 0:6, :])
                nc.scalar.copy(out=qT[0:32, 6, :], in_=qt_ps[0:32, 6, :])

                # qw^T = q^T * w_cq (per-partition scalars); column 64 holds
                # w_c so the S~ matmul also emits sc = c.w_c as PSUM row 64
                # (engine PSUM reads must start at a multiple-of-32 partition).
                qwT = ex.tile([128, DC, 65], R32)
                for k in range(DC):
                    sz = DSZ[k]
                    nc.scalar.activation(
                        out=qwT[0:sz, k, 0:QLEN], in_=qT[0:sz, k, :],
                        func=mybir.ActivationFunctionType.Copy,
                        scale=wcol[0:sz, 14 + k : 15 + k],
                    )
                    nc.gpsimd.tensor_copy(
                        out=qwT[0:sz, k, QLEN:64], in_=zeros_f[0:sz, :]
                    )
                    nc.gpsimd.tensor_copy(
                        out=qwT[0:sz, k, 64:65], in_=wcol[0:sz, k : k + 1]
                    )

                # ---- c^T (d-major) via PE transposes ----
                cT = ex.tile([128, DC, CLEN], R32)
                for k in range(DC):
                    sz = DSZ[k]
                    ct_ps = ps_small.tile([128, CLEN], F32, tag="ps_small")
                    for ch in range(IC):
                        nc.tensor.transpose(
                            out=ct_ps[0:sz, ch * IP : (ch + 1) * IP],
                            in_=c_nat[:, ch, k * 128 : k * 128 + sz],
                            identity=ident[0:IP, 0:IP],
                        )
                    nc.scalar.copy(out=cT[0:sz, k, :], in_=ct_ps[0:sz, :])

                # ---- sq[j] = q_j.w_q  (1,50) ----
                sq_ps = ps_small.tile([1, QLEN], F32, tag="ps_small")
                for k in range(DC):
                    sz = DSZ[k]
                    nc.tensor.matmul(
                        out=sq_ps,
                        lhsT=wcol_r[0:sz, 7 + k : 8 + k],
                        rhs=qT[0:sz, k, :],
                        start=(k == 0),
                        stop=(k == DC - 1),
                    )
                sq_row = ex.tile([1, QLEN], R32)
                nc.scalar.copy(out=sq_row, in_=sq_ps)

                # ---- S^T (+ sc row 64) = [qw|0|w_c]^T.T @ c^T  (65, 400) ----
                st_ps = ps_small.tile([65, CLEN], F32, tag="ps_small")
                for k in range(DC):
                    sz = DSZ[k]
                    nc.tensor.matmul(
                        out=st_ps,
                        lhsT=qwT[0:sz, k, :],
                        rhs=cT[0:sz, k, :],
                        start=(k == 0),
                        stop=(k == DC - 1),
                    )
                sc_row = ex.tile([1, CLEN], R32)
                nc.scalar.copy(out=sc_row, in_=st_ps[64:65, :])
                # rank-1 bias terms accumulated onto the closed S~ group
                nc.tensor.matmul(
                    out=st_ps[0:QLEN, :], lhsT=ones50, rhs=sc_row,
                    start=False, stop=False, skip_group_check=True,
                )
                nc.tensor.matmul(
                    out=st_ps[0:QLEN, :], lhsT=sq_row, rhs=ones_c,
                    start=False, stop=True, skip_group_check=True,
                )
                St = ex.tile([QLEN, CLEN], F32)
                nc.scalar.copy(out=St, in_=st_ps[0:QLEN, :])

                # ---- S1^T = softmax over free dim of S^T ----
                m1 = ex.tile([QLEN, 1], F32)
                nc.vector.tensor_reduce(
                    out=m1, in_=St, axis=mybir.AxisListType.X,
                    op=mybir.AluOpType.max, negate=True,
                )
                e1 = ex.tile([QLEN, CLEN], R32)
                s1sum = ex.tile([QLEN, 1], F32)
                nc.scalar.activation(
                    out=e1, in_=St, func=mybir.ActivationFunctionType.Exp,
                    bias=m1, scale=1.0, accum_out=s1sum,
                )
                r1 = ex.tile([QLEN, 1], F32)
                nc.vector.reciprocal(out=r1, in_=s1sum)
                # e1 stays unnormalized; r1 is folded into A's rhs (rq) and
                # Bv's rhs (T) below.
                rq = ex.tile([QLEN, D], R32)
                nc.scalar.activation(
                    out=rq, in_=q_nat,
                    func=mybir.ActivationFunctionType.Copy, scale=r1,
                )

                # ---- S natural (PE transpose) + softmax over j -> S2 ----
                sn_ps = ps_small.tile([IP, IC, QLEN], F32, tag="ps_small")
                for ch in range(IC):
                    nc.tensor.transpose(
                        out=sn_ps[:, ch, :],
                        in_=St[:, ch * IP : (ch + 1) * IP],
                        identity=ident[0:QLEN, 0:QLEN],
                    )
                s2 = ex.tile([IP, IC, QLEN], F32)
                s2f = ex.tile([IP, IC, QLEN], F32)
                m2 = ex.tile([IP, IC], F32)
                s2sum = ex.tile([IP, IC], F32)
                r2 = ex.tile([IP, IC], F32)
                for ch in range(IC):
                    nc.vector.tensor_reduce(
                        out=m2[:, ch : ch + 1], in_=sn_ps[:, ch, :],
                        axis=mybir.AxisListType.X, op=mybir.AluOpType.max,
                        negate=True,
                    )
                    nc.scalar.activation(
                        out=s2f[:, ch, :], in_=sn_ps[:, ch, :],
                        func=mybir.ActivationFunctionType.Exp,
                        bias=m2[:, ch : ch + 1], scale=1.0,
                        accum_out=s2sum[:, ch : ch + 1],
                    )
                    nc.vector.reciprocal(
                        out=r2[:, ch : ch + 1], in_=s2sum[:, ch : ch + 1]
                    )
                    nc.scalar.activation(
                        out=s2[:, ch, :], in_=s2f[:, ch, :],
                        func=mybir.ActivationFunctionType.Copy,
                        scale=r2[:, ch : ch + 1],
                    )

                # ---- T = S2^T @ c  (50, 800) ----
                T_sb = ex.tile([QLEN, D], R32)
                for h in range(NH):
                    t_ps = ps_big.tile([IP, HALF], F32, tag="ps_big")
                    for ch in range(IC):
                        nc.tensor.matmul(
                            out=t_ps[0:QLEN, :],
                            lhsT=s2[:, ch, :],
                            rhs=c_nat[:, ch, h * HALF : (h + 1) * HALF],
                            start=(ch == 0),
                            stop=(ch == IC - 1),
                        )
                    nc.scalar.activation(
                        out=T_sb[:, h * HALF : (h + 1) * HALF], in_=t_ps[0:QLEN, :],
                        func=mybir.ActivationFunctionType.Copy, scale=r1,
                    )

                # ---- per i-chunk: A, Bv, elementwise, store ----
                for ch in range(IC):
                    o_sb = osb_pool.tile([IP, 3 * D], F32, tag="osb")
                    a_ps = []
                    bv_ps = []
                    for h in range(NH):
                        ap = ps_big.tile([IP, HALF], F32, tag="ps_big")
                        nc.tensor.matmul(
                            out=ap,
                            lhsT=e1[:, ch * IP : (ch + 1) * IP],
                            rhs=rq[:, h * HALF : (h + 1) * HALF],
                            start=True, stop=True,
                        )
                        a_ps.append(ap)
                    for h in range(NH):
                        bp = ps_big.tile([IP, HALF], F32, tag="ps_big")
                        nc.tensor.matmul(
                            out=bp,
                            lhsT=e1[:, ch * IP : (ch + 1) * IP],
                            rhs=T_sb[:, h * HALF : (h + 1) * HALF],
                            start=True, stop=True,
                        )
                        bv_ps.append(bp)
                    for h in range(NH):
                        sl = slice(h * HALF, (h + 1) * HALF)
                        nc.vector.tensor_copy(
                            out=o_sb[:, h * HALF : (h + 1) * HALF], in_=a_ps[h]
                        )
                        nc.vector.tensor_mul(
                            o_sb[:, D + h * HALF : D + (h + 1) * HALF],
                            c_nat[:, ch, sl], a_ps[h],
                        )
                        nc.vector.tensor_mul(
                            o_sb[:, 2 * D + h * HALF : 2 * D + (h + 1) * HALF],
                            c_nat[:, ch, sl], bv_ps[h],
                        )
                    nc.gpsimd.dma_start(
                        out=out[b, ch * IP : (ch + 1) * IP, 0:D], in_=c_nat[:, ch, :]
                    )
                    nc.gpsimd.dma_start(
                        out=out[b, ch * IP : (ch + 1) * IP, D : 4 * D], in_=o_sb
                    )

    nc.compile()
    return nc


_NC = None


def _get_nc():
    global _NC
    if _NC is None:
        _NC = build_nc()
    return _NC


def run(inputs: dict, trace: bool = False, **kw):
    nc = _get_nc()
    c, q, w0 = inputs["c"], inputs["q"], inputs["W0"]
    in_maps = [
        {
            "c": np.ascontiguousarray(c[i * BSH : (i + 1) * BSH]),
            "q": np.ascontiguousarray(q[i * BSH : (i + 1) * BSH]),
            "W0": np.ascontiguousarray(w0),
        }
        for i in range(NCORES)
    ]
    res = run_bass_kernel_spmd(
        nc, in_maps, core_ids=list(range(NCORES)), trace=trace, **kw
    )
    outs = np.concatenate([r["out"] for r in res.results], axis=0)
    return outs, res


def kernel(**inputs) -> np.ndarray:
    out, _ = run(inputs, trace=False)
    return out
